# revision 4
# baseline (speedup 1.0000x reference)
"""Trainium2 Bass kernel for nn_Attention_38182259261827.

Multi-head attention (B=4, C=512, L=2048, H=8, D=64) with pointwise-conv
QKV / output projections.

Sharding: core c handles batch b=c//2, head-group g=c%2 (4 heads each).
Each core computes its partial output-projection sum over its 4 heads;
the two partials per batch are summed host-side (plus bias), so no
on-device collective is needed.

Per-core pipeline (all matmuls bf16, f32 PSUM accumulation):
  1. qk projection: qk[f=512, l=2048], f = 4 heads x (q|k) packed.
  2. v^T projection: va[l, 4*65] = per-head [v^T | ones]; the ones column
     makes the PV matmul emit the softmax row-sums for free.
  3. Per (i-tile, head): S^T[j, i] = k^T q on PE; exp(S*scale) on ACT
     (no max-subtract needed: S ~ N(0,1)); PV accumulation in PSUM gives
     O[i, 0:64] and row-sum s_i at col 64; DVE reciprocal + scale; PE
     transpose to O^T; output projection per i-tile.
"""

import sys

if "/opt/trn_rl_repo" not in sys.path:
    sys.path.insert(0, "/opt/trn_rl_repo")

import numpy as np

import concourse.bass as bass
import concourse.mybir as mybir
from concourse import bacc
from concourse.tile import TileContext
from concourse.bass_utils import run_bass_kernel_spmd
from concourse.masks import make_identity

F32 = mybir.dt.float32
BF16 = mybir.dt.bfloat16
EXP = mybir.ActivationFunctionType.Exp

B, C, L = 4, 512, 2048
HEADS, D = 8, 64
HL = HEADS // 2          # 4 local heads per core
SCALE = D ** -0.5        # 0.125
N_CORES = 8
NKC = C // 128           # 4 contraction chunks
NLT = L // 512           # 4 l/i tiles of 512
NLC = L // 128           # 16 l/j chunks of 128
J_GROUPS = [[0, 1, 2], [3, 4, 5], [6, 7, 8], [9, 10, 11], [12, 13, 14], [15]]

_NC_CACHE = None


def _build_nc():
    nc = bacc.Bacc(
        "TRN2",
        target_bir_lowering=False,
        debug=False,
        enable_asserts=False,
        num_devices=N_CORES,
    )
    x_d = nc.dram_tensor("x", [C, L], F32, kind="ExternalInput")
    wqk_d = nc.dram_tensor("wqk", [C, 512], F32, kind="ExternalInput")
    wv_d = nc.dram_tensor("wv", [C, 256], F32, kind="ExternalInput")
    wo_d = nc.dram_tensor("wout", [256, C], F32, kind="ExternalInput")
    out_d = nc.dram_tensor("out", [C, L], F32, kind="ExternalOutput")

    with TileContext(nc) as tc:
        with (
            tc.tile_pool(name="sb", bufs=1) as SB,
            tc.tile_pool(name="ps", space="PSUM", bufs=1) as PS,
        ):
            ident = SB.tile([128, 128], BF16, tag="ident", bufs=1, name="ident")
            make_identity(nc, ident)

            # --- weights: DMA + cast to bf16 ---
            wqk_bf = [
                SB.tile([128, 512], BF16, tag=f"wqk{i}", bufs=1, name=f"wqk{i}")
                for i in range(NKC)
            ]
            wv_bf = [
                SB.tile([128, 256], BF16, tag=f"wv{i}", bufs=1, name=f"wv{i}")
                for i in range(NKC)
            ]
            wo_bf = [
                SB.tile([128, 512], BF16, tag=f"wo{i}", bufs=1, name=f"wo{i}")
                for i in range(2)
            ]
            for i in range(NKC):
                wf = SB.tile([128, 512], F32, tag="wstage", bufs=2, name=f"wqkf{i}")
                nc.sync.dma_start(out=wf, in_=wqk_d[i * 128:(i + 1) * 128, :])
                nc.vector.tensor_copy(wqk_bf[i], wf)
            for i in range(NKC):
                wf = SB.tile([128, 256], F32, tag="wstagev", bufs=2, name=f"wvf{i}")
                nc.sync.dma_start(out=wf, in_=wv_d[i * 128:(i + 1) * 128, :])
                nc.vector.tensor_copy(wv_bf[i], wf)
            for i in range(2):
                wf = SB.tile([128, 512], F32, tag="wstage", bufs=2, name=f"wof{i}")
                nc.sync.dma_start(out=wf, in_=wo_d[i * 128:(i + 1) * 128, :])
                nc.vector.tensor_copy(wo_bf[i], wf)

            # --- x: DMA + cast to bf16 (on ACT, idle before exps) ---
            x_bf = [
                SB.tile([128, L], BF16, tag=f"x{i}", bufs=1, name=f"x{i}")
                for i in range(NKC)
            ]
            for i in range(NKC):
                xf = SB.tile([128, L], F32, tag="xstage", bufs=2, name=f"xf{i}")
                nc.sync.dma_start(out=xf, in_=x_d[i * 128:(i + 1) * 128, :])
                nc.scalar.copy(x_bf[i], xf)

            # --- qk projection: qk_bf[m][f 128, l 2048], m-chunks:
            #     m=0: q_h0|q_h1, m=1: q_h2|q_h3, m=2: k_h0|k_h1, m=3: k_h2|k_h3
            qk_bf = [
                SB.tile([128, L], BF16, tag=f"qk{m}", bufs=1, name=f"qk{m}")
                for m in range(4)
            ]
            for m in range(4):
                for lt in range(NLT):
                    pp = PS.tile([128, 512], F32, tag="st", bufs=2, name=f"ppqk{m}_{lt}")
                    for kc in range(NKC):
                        nc.tensor.matmul(
                            pp,
                            wqk_bf[kc][:, m * 128:(m + 1) * 128],
                            x_bf[kc][:, lt * 512:(lt + 1) * 512],
                            start=(kc == 0),
                            stop=(kc == NKC - 1),
                        )
                    nc.vector.tensor_copy(qk_bf[m][:, lt * 512:(lt + 1) * 512], pp)

            # --- v^T projection into va[l-chunk partitions, (lc*4+h), 0:64],
            #     col 64 of each head's block stays 1.0 (softmax row-sums)
            va = SB.tile([128, 4 * NLC, 65], BF16, tag="va", bufs=1, name="va")
            nc.vector.memset(va, 1.0)
            for lc in range(NLC):
                vp = PS.tile([128, 256], F32, tag="st", bufs=2, name=f"vp{lc}")
                for kc in range(NKC):
                    nc.tensor.matmul(
                        vp,
                        x_bf[kc][:, lc * 128:(lc + 1) * 128],
                        wv_bf[kc],
                        start=(kc == 0),
                        stop=(kc == NKC - 1),
                    )
                nc.vector.tensor_copy(
                    va[:, lc * 4:(lc + 1) * 4, 0:64],
                    vp.rearrange("p (h d) -> p h d", h=4),
                )

            # --- attention + output projection, i-tile outer ---
            OT_bf = [
                SB.tile([128, L], BF16, tag=f"otb{i}", bufs=1, name=f"otb{i}")
                for i in range(2)
            ]
            for it in range(NLT):
                for h in range(HL):
                    qp = 64 * (h % 2)
                    q_ap = qk_bf[h // 2][qp:qp + 64, it * 512:(it + 1) * 512]
                    o_ps = PS.tile([128, 4, 65], F32, tag="o", bufs=1, name=f"o{it}_{h}")
                    for grp in J_GROUPS:
                        st = PS.tile(
                            [128, 3, 512], F32, tag="st", bufs=2, name=f"st{it}_{h}"
                        )
                        for gi, jc in enumerate(grp):
                            k_ap = qk_bf[2 + h // 2][qp:qp + 64, jc * 128:(jc + 1) * 128]
                            nc.tensor.matmul(
                                st[:, gi, :], k_ap, q_ap, start=True, stop=True
                            )
                        g = len(grp)
                        ex = SB.tile(
                            [128, 3, 512], BF16, tag="exp", bufs=3, name=f"ex{it}_{h}"
                        )
                        nc.scalar.activation(
                            ex[:, 0:g, :], st[:, 0:g, :], EXP, scale=float(SCALE)
                        )
                        for gi, jc in enumerate(grp):
                            rhs = va[:, jc * 4 + h, :]  # [128, 65]
                            for s in range(4):
                                # start=True clears has_written for the WHOLE
                                # bank, so only the first of the four
                                # interleaved chains may set it; the others
                                # begin via overwrite-on-cleared-bit.
                                nc.tensor.matmul(
                                    o_ps[:, s, :],
                                    ex[:, gi, s * 128:(s + 1) * 128],
                                    rhs,
                                    start=(jc == 0 and s == 0),
                                    stop=(jc == NLC - 1),
                                )
                    # normalize rows by the sums in col 64, transpose to O^T
                    otp = PS.tile([64, 4, 128], BF16, tag="y", bufs=1, name=f"otp{it}_{h}")
                    for s in range(4):
                        rs = SB.tile([128, 1], F32, tag="rs", bufs=2, name=f"rs{s}")
                        nc.vector.reciprocal(rs, o_ps[:, s, 64:65])
                        osb = SB.tile([128, 64], BF16, tag="osb", bufs=2, name=f"osb{s}")
                        nc.vector.tensor_scalar_mul(osb, o_ps[:, s, 0:64], rs)
                        nc.tensor.transpose(otp[:, s, :], osb, ident)
                    cp = 64 * (h % 2)
                    nc.vector.tensor_copy(
                        OT_bf[h // 2][cp:cp + 64, it * 512:(it + 1) * 512],
                        otp.rearrange("p s f -> p (s f)"),
                    )
                # output projection for l-tile == it
                for fm in range(4):
                    yp = PS.tile([128, 512], F32, tag="y", bufs=1, name=f"yp{it}_{fm}")
                    for kc in range(2):
                        nc.tensor.matmul(
                            yp,
                            wo_bf[kc][:, fm * 128:(fm + 1) * 128],
                            OT_bf[kc][:, it * 512:(it + 1) * 512],
                            start=(kc == 0),
                            stop=(kc == 1),
                        )
                    ysb = SB.tile([128, 512], F32, tag="ysb", bufs=2, name=f"ysb{it}_{fm}")
                    nc.vector.tensor_copy(ysb, yp)
                    nc.sync.dma_start(
                        out=out_d[fm * 128:(fm + 1) * 128, it * 512:(it + 1) * 512],
                        in_=ysb,
                    )
    nc.compile()
    return nc


def _shard_inputs(x, w_qkv, w_out):
    """Per-core input maps: core c = (batch c//2, head-group c%2)."""
    in_maps = []
    for c in range(N_CORES):
        b, g = c // 2, c % 2
        cols = slice(g * 256, (g + 1) * 256)
        wqk_c = np.ascontiguousarray(
            np.concatenate([w_qkv[:, 0:512][:, cols], w_qkv[:, 512:1024][:, cols]], axis=1)
        )
        wv_c = np.ascontiguousarray(w_qkv[:, 1024:1536][:, cols])
        wo_c = np.ascontiguousarray(w_out[g * 256:(g + 1) * 256, :])
        in_maps.append(
            {
                "x": np.ascontiguousarray(x[b]),
                "wqk": wqk_c,
                "wv": wv_c,
                "wout": wo_c,
            }
        )
    return in_maps


def _run(x, w_qkv, w_out, b_out, trace=False, tmpdir=None):
    global _NC_CACHE
    if _NC_CACHE is None:
        _NC_CACHE = _build_nc()
    nc = _NC_CACHE
    in_maps = _shard_inputs(
        np.asarray(x, np.float32),
        np.asarray(w_qkv, np.float32),
        np.asarray(w_out, np.float32),
    )
    res = run_bass_kernel_spmd(
        nc, in_maps, core_ids=list(range(N_CORES)), trace=trace, tmpdir=tmpdir
    )
    b_out = np.asarray(b_out, np.float32)
    y = np.empty((B, C, L), np.float32)
    for b in range(B):
        y[b] = res.results[2 * b]["out"] + res.results[2 * b + 1]["out"] + b_out[:, None]
    return y, res


def kernel(x, w_qkv, w_out, b_out):
    y, _ = _run(x, w_qkv, w_out, b_out, trace=False)
    return y


if __name__ == "__main__":
    rng = np.random.default_rng(0)
    x = rng.standard_normal((B, C, L)).astype(np.float32)
    w_qkv = (rng.standard_normal((C, 3 * 512)) * C ** -0.5).astype(np.float32)
    w_out = (rng.standard_normal((512, C)) * 512 ** -0.5).astype(np.float32)
    b_out = np.zeros((C,), np.float32)
    y = kernel(x=x, w_qkv=w_qkv, w_out=w_out, b_out=b_out)
    print("ran ok", y.shape, y.dtype)
